# revision 9
# baseline (speedup 1.0000x reference)
"""AMPNNConv distributed Trainium2 kernel.

Math (reformulated from the reference, numerically equivalent):
    w_m = efeat @ W_msg + b_msg          [E, 16*16]
    w_a = efeat @ W_attn + b_attn        [E, 16*16]
    h   = feat[src]                      [E, 16]   (broadcast over out dim)
    ex  = exp(w_a * h)                   [E, 16, 16]   (no max-subtraction:
                                          |w_a*h| is small, exp is f32-safe;
                                          softmax is shift-invariant)
    num[n] = sum_{e: dst[e]=n} (w_m*h) * ex
    den[n] = sum_{e: dst[e]=n} ex
    out[n, j] = sum_i num[n,i,j] / den[n,i,j]

Sharding: edges sorted by dst on host; core c owns nodes
[c*3750, (c+1)*3750) so every segment-sum is core-local (no collectives).
Within a core, edges are bucketed into 30 windows of 128 destination
nodes; each 128-edge tile targets one window and the segment-sum is a
one-hot matmul accumulated in PSUM.
"""

import numpy as np

N_NODES = 30000
N_EDGES = 300000
F = 16              # in = out = edge_dim
C = 8               # cores
NPC = N_NODES // C  # nodes per core = 3750
P = 128
W = (NPC + P - 1) // P  # windows per core = 30
NPAD = W * P            # padded nodes per core = 3840
KA = 17                 # edge_dim + 1 (bias row)


def _prep(feat, efeat, W_msg, b_msg, W_attn, b_attn, src, dst):
    """Host-side shard/sort/pad. Returns (in_maps, Tw, T)."""
    f32 = np.float32
    order = np.argsort(dst, kind="stable")
    dsts = dst[order].astype(np.int64)
    core_of = dsts // NPC
    nloc = dsts - core_of * NPC
    win = nloc // P
    wloc = nloc % P

    # tiles per window: max over cores, >= 1
    cnt = np.zeros((C, W), np.int64)
    np.add.at(cnt, (core_of, win), 1)
    Tw = np.maximum(1, -(-cnt.max(axis=0) // P)).astype(np.int64)
    T = int(Tw.sum())
    wb = np.concatenate([[0], np.cumsum(Tw)])[:-1]  # window tile base

    # [17, 512] combined weights with bias row
    Wcat = np.concatenate(
        [np.concatenate([W_msg, W_attn], axis=1),
         np.concatenate([b_msg, b_attn])[None, :]], axis=0).astype(f32)

    in_maps = []
    for c in range(C):
        m = core_of == c
        e_idx = order[m]          # original edge ids, sorted by local node
        w_c = win[m]
        wl = wloc[m]
        cc = cnt[c]
        run_starts = np.concatenate([[0], np.cumsum(cc)])[:-1]
        rank = np.arange(m.sum()) - np.repeat(run_starts, cc)
        slot = (wb[w_c] * P + rank).astype(np.int64)

        efT = np.zeros((KA, T * P), f32)
        h = np.zeros((T * P, F), f32)
        dstf = np.full((T * P, 1), -1.0, f32)
        efT[:F, slot] = efeat[e_idx].T
        efT[F, slot] = 1.0
        h[slot] = feat[src[e_idx]]
        dstf[slot, 0] = wl
        in_maps.append({"efeatT": efT, "h": h, "dstf": dstf, "wcat": Wcat})
    return in_maps, Tw, T


def _build(Tw, T):
    import concourse.bass as bass
    import concourse.mybir as mybir
    from concourse import bacc, tile

    f32 = mybir.dt.float32
    bf16 = mybir.dt.bfloat16
    i32 = mybir.dt.int32
    mult = mybir.AluOpType.mult

    nc = bacc.Bacc(None, target_bir_lowering=False)
    ef_ext = nc.declare_dram_parameter("efeatT", [KA, T * P], f32, isOutput=False)
    h_ext = nc.declare_dram_parameter("h", [T * P, F], f32, isOutput=False)
    d_ext = nc.declare_dram_parameter("dstf", [T * P, 1], f32, isOutput=False)
    w_ext = nc.declare_dram_parameter("wcat", [KA, 4 * P], f32, isOutput=False)
    out_ext = nc.declare_dram_parameter("out", [NPAD, F], f32, isOutput=True)

    CH = 8  # tiles per DMA chunk

    with tile.TileContext(nc) as tc:
        with (
            tc.tile_pool(name="const", bufs=1) as constp,
            tc.tile_pool(name="chunk", bufs=3) as chunkp,
            tc.tile_pool(name="work", bufs=4) as workp,
            tc.tile_pool(name="evac", bufs=2) as evacp,
            tc.tile_pool(name="wps", bufs=2, space=bass.MemorySpace.PSUM) as wpsp,
            tc.tile_pool(name="acc", bufs=2, space=bass.MemorySpace.PSUM) as accp,
        ):
            wcat = constp.tile([KA, 4 * P], f32, tag="wcat")
            nc.sync.dma_start(wcat[:], w_ext[:])
            iota_i = constp.tile([P, P], i32, tag="ioti")
            nc.gpsimd.iota(iota_i[:], pattern=[[1, P]], base=0,
                           channel_multiplier=0)
            iota_f = constp.tile([P, P], f32, tag="iotf")
            nc.vector.tensor_copy(iota_f[:], iota_i[:])
            eps = constp.tile([P, 1], f32, tag="eps")
            nc.vector.memset(eps[:], 1e-30)

            n_chunks = (T + CH - 1) // CH
            ef_ch = [None] * n_chunks
            h_ch = [None] * n_chunks
            d_ch = [None] * n_chunks

            def load_chunk(ci):
                t0 = ci * CH
                n = min(CH, T - t0)
                ef = chunkp.tile([KA, CH * P], f32, tag="efch")
                nc.sync.dma_start(ef[:, :n * P],
                                  ef_ext[:, t0 * P:(t0 + n) * P])
                hh = chunkp.tile([P, CH, F], f32, tag="hch")
                nc.sync.dma_start(
                    hh[:, :n, :],
                    h_ext[t0 * P:(t0 + n) * P, :].rearrange(
                        "(a p) f -> p a f", p=P))
                dd = chunkp.tile([P, CH, 1], f32, tag="dch")
                nc.sync.dma_start(
                    dd[:, :n, :],
                    d_ext[t0 * P:(t0 + n) * P, :].rearrange(
                        "(a p) o -> p a o", p=P))
                ef_ch[ci], h_ch[ci], d_ch[ci] = ef, hh, dd

            t = 0
            for w in range(W):
                acc = accp.tile([P, 4 * P], f32, tag="acc")
                for s in range(int(Tw[w])):
                    ci, si = divmod(t, CH)
                    if ef_ch[ci] is None:
                        load_chunk(ci)
                    ef_t = ef_ch[ci][:, si * P:(si + 1) * P]
                    h_t = h_ch[ci][:, si, :]
                    d_t = d_ch[ci][:, si, :]

                    wps = wpsp.tile([P, 4 * P], f32, tag="wps")
                    nc.tensor.matmul(wps[:], ef_t, wcat[:],
                                     start=True, stop=True,
                                     skip_group_check=True)
                    # e12 = wps * h (h broadcast over halves and j)
                    e12 = workp.tile([P, 4 * P], bf16, tag="e12")
                    h_b = h_t.unsqueeze(1).unsqueeze(3).broadcast_to(
                        [P, 2, F, F])
                    nc.vector.tensor_tensor(
                        e12[:].rearrange("p (a i j) -> p a i j", a=2, i=F),
                        wps[:].rearrange("p (a i j) -> p a i j", a=2, i=F),
                        h_b, mult)
                    pay = workp.tile([P, 4 * P], bf16, tag="pay")
                    nc.scalar.activation(pay[:, 2 * P:], e12[:, 2 * P:],
                                         mybir.ActivationFunctionType.Exp)
                    nc.gpsimd.tensor_tensor(pay[:, :2 * P], e12[:, :2 * P],
                                            pay[:, 2 * P:], mult)
                    oh = workp.tile([P, P], bf16, tag="oh")
                    nc.vector.tensor_scalar(oh[:], iota_f[:], d_t, None,
                                            mybir.AluOpType.is_equal)
                    nc.tensor.matmul(acc[:], oh[:], pay[:],
                                     start=(s == 0), stop=(s == int(Tw[w]) - 1),
                                     skip_group_check=True)
                    if si == CH - 1:
                        ef_ch[ci] = h_ch[ci] = d_ch[ci] = None
                    t += 1
                # evacuate window: out[n,j] = sum_i num/den
                logd = evacp.tile([P, 2 * P], f32, tag="logd")
                nc.scalar.activation(logd[:], acc[:, 2 * P:],
                                     mybir.ActivationFunctionType.Ln,
                                     bias=eps[:])
                rden = evacp.tile([P, 2 * P], f32, tag="rden")
                nc.scalar.activation(rden[:], logd[:],
                                     mybir.ActivationFunctionType.Exp,
                                     scale=-1.0)
                ft = evacp.tile([P, 2 * P], f32, tag="ft")
                nc.vector.tensor_tensor(ft[:], acc[:, :2 * P], rden[:], mult)
                outw = evacp.tile([P, F], f32, tag="outw")
                nc.vector.tensor_reduce(
                    outw[:],
                    ft[:].rearrange("p (i j) -> p j i", i=F),
                    mybir.AxisListType.X, mybir.AluOpType.add)
                nc.sync.dma_start(out_ext[w * P:(w + 1) * P, :], outw[:])
    nc.compile()
    return nc


TRACE = False          # set True (e.g. from test.py) to capture a profile
TRACE_DIR = None       # where to keep NTFF/perfetto artifacts
LAST_RESULT = None     # BassKernelResults of the last run (for profiling)


def kernel(feat, efeat, W_msg, b_msg, W_attn, b_attn, src, dst):
    global LAST_RESULT
    from concourse.bass_utils import run_bass_kernel_spmd

    in_maps, Tw, T = _prep(feat, efeat, W_msg, b_msg, W_attn, b_attn,
                           src, dst)
    nc = _build(Tw, T)
    res = run_bass_kernel_spmd(nc, in_maps, core_ids=list(range(C)),
                               trace=TRACE, tmpdir=TRACE_DIR)
    LAST_RESULT = res
    out = np.empty((N_NODES, F), np.float32)
    for c in range(C):
        out[c * NPC:(c + 1) * NPC] = res.results[c]["out"][:NPC]
    return out


# revision 18
# speedup vs baseline: 2.4813x; 2.4813x over previous
"""AMPNNConv distributed Trainium2 kernel.

Math (reformulated from the reference, numerically equivalent):
    w_m = efeat @ W_msg + b_msg          [E, 16*16]
    w_a = efeat @ W_attn + b_attn        [E, 16*16]
    h   = feat[src]                      [E, 16]   (broadcast over out dim)
    ex  = exp(w_a * h)                   (no max-subtraction: |w_a*h| <~ 8,
                                          exp is f32-safe; softmax is
                                          shift-invariant)
    num[n] = sum_{e: dst[e]=n} (w_m*h) * ex
    den[n] = sum_{e: dst[e]=n} ex
    out[n, j] = sum_i num[n,i,j] / den[n,i,j]

Sharding: edges sorted by dst on host; core c owns nodes
[c*3750, (c+1)*3750) so every segment-sum is core-local (no collectives).
Within a core, edges are bucketed into 30 windows of 128 destination
nodes; each 128-edge tile targets one window and the segment-sum is a
one-hot matmul accumulated in PSUM.

Performance notes (measured on TRN2):
- bf16 matmuls with K padded to 128 stream at ~216ns/512 cols; K<128
  runs 2x slower, f32 runs LOW_HIGH 4-pass.  The K-pad is free: only the
  constant Wcat operand needs zeros in rows 17..127 (garbage * 0 = 0),
  so the per-tile efeatT SBUF rows 17..127 are never written.
- LDWEIGHTS overlaps with in-flight matmuls (64-deep PE queue).
- Elementwise ops process 2 tiles per instruction to amortize the
  per-op overhead (SBUF read-write bubble errata).
- num/den are stashed to SBUF per window; Ln/Exp division runs once at
  the end (2 ACT table loads instead of 2 per window).
"""

import numpy as np

N_NODES = 30000
N_EDGES = 300000
F = 16              # in = out = edge_dim
C = 8               # cores
NPC = N_NODES // C  # nodes per core = 3750
P = 128
W = (NPC + P - 1) // P  # windows per core = 30
NPAD = W * P            # padded nodes per core = 3840
KA = 17                 # edge_dim + 1 (bias row)


def _prep(feat, efeat, W_msg, b_msg, W_attn, b_attn, src, dst):
    """Host-side shard/sort/pad. Returns (in_maps, Tw, T)."""
    import ml_dtypes
    f32 = np.float32
    bf16 = ml_dtypes.bfloat16
    order = np.argsort(dst, kind="stable")
    dsts = dst[order].astype(np.int64)
    core_of = dsts // NPC
    nloc = dsts - core_of * NPC
    win = nloc // P
    wloc = nloc % P

    # tiles per window: max over cores, >= 1
    cnt = np.zeros((C, W), np.int64)
    np.add.at(cnt, (core_of, win), 1)
    Tw = np.maximum(1, -(-cnt.max(axis=0) // P)).astype(np.int64)
    if Tw.sum() % 2:
        Tw[-1] += 1          # keep T even (elementwise ops pair tiles)
    T = int(Tw.sum())
    wb = np.concatenate([[0], np.cumsum(Tw)])[:-1]  # window tile base

    # [128, 512] combined weights: rows 0..15 = W, row 16 = bias,
    # rows 17..127 = zeros (K-pad so matmuls run at K=128 speed)
    Wcat = np.zeros((P, 4 * P), f32)
    Wcat[:F] = np.concatenate([W_msg, W_attn], axis=1)
    Wcat[F] = np.concatenate([b_msg, b_attn])

    in_maps = []
    for c in range(C):
        m = core_of == c
        e_idx = order[m]          # original edge ids, sorted by local node
        w_c = win[m]
        wl = wloc[m]
        cc = cnt[c]
        run_starts = np.concatenate([[0], np.cumsum(cc)])[:-1]
        rank = np.arange(m.sum()) - np.repeat(run_starts, cc)
        slot = (wb[w_c] * P + rank).astype(np.int64)

        efT = np.zeros((KA, T * P), f32)
        h = np.zeros((T * P, F), f32)
        dstf = np.full((T * P, 1), -1.0, f32)
        efT[:F, slot] = efeat[e_idx].T
        efT[F, slot] = 1.0
        h[slot] = feat[src[e_idx]]
        dstf[slot, 0] = wl
        in_maps.append({"efeatT": efT.astype(bf16), "h": h,
                        "dstf": dstf,
                        "wcat": Wcat.astype(bf16)})
    return in_maps, Tw, T


def _build(Tw, T):
    import concourse.bass as bass
    import concourse.mybir as mybir
    from concourse import bacc, tile

    f32 = mybir.dt.float32
    bf16 = mybir.dt.bfloat16
    i32 = mybir.dt.int32
    mult = mybir.AluOpType.mult

    nc = bacc.Bacc(None, target_bir_lowering=False)
    ef_ext = nc.declare_dram_parameter("efeatT", [KA, T * P], bf16,
                                       isOutput=False)
    h_ext = nc.declare_dram_parameter("h", [T * P, F], f32, isOutput=False)
    d_ext = nc.declare_dram_parameter("dstf", [T * P, 1], f32,
                                      isOutput=False)
    w_ext = nc.declare_dram_parameter("wcat", [P, 4 * P], bf16,
                                      isOutput=False)
    out_ext = nc.declare_dram_parameter("out", [NPAD, F], f32, isOutput=True)

    CH = 8  # tiles per DMA chunk (must be even)

    with tile.TileContext(nc) as tc:
        with (
            tc.tile_pool(name="const", bufs=1) as constp,
            tc.tile_pool(name="chunk", bufs=3) as chunkp,
            tc.tile_pool(name="work", bufs=4) as workp,
            tc.tile_pool(name="stash", bufs=1) as stashp,
            tc.tile_pool(name="tail", bufs=1) as tailp,
            tc.tile_pool(name="wps", bufs=2, space=bass.MemorySpace.PSUM) as wpsp,
            tc.tile_pool(name="acc", bufs=2, space=bass.MemorySpace.PSUM) as accp,
        ):
            wcat = constp.tile([P, 4 * P], bf16, tag="wcat")
            nc.sync.dma_start(wcat[:], w_ext[:])
            iota_i = constp.tile([P, P], i32, tag="ioti")
            nc.gpsimd.iota(iota_i[:], pattern=[[1, P]], base=0,
                           channel_multiplier=0)
            iota_b = constp.tile([P, P], bf16, tag="iotb")
            nc.vector.tensor_copy(iota_b[:], iota_i[:])
            eps = constp.tile([P, 1], f32, tag="eps")
            nc.vector.memset(eps[:], 1e-30)

            # per-window num/den stash in SBUF (f32)
            nums = stashp.tile([P, W, 2 * P], f32, tag="nums")
            dens = stashp.tile([P, W, 2 * P], f32, tag="dens")

            n_chunks = (T + CH - 1) // CH
            ef_ch = [None] * n_chunks
            h_ch = [None] * n_chunks
            d_ch = [None] * n_chunks

            def load_chunk(ci):
                t0 = ci * CH
                n = min(CH, T - t0)
                # rows 17..127 must be finite (NaN*0=NaN in the matmul);
                # values are irrelevant since Wcat rows 17..127 are zero
                ef = chunkp.tile([P, CH * P], bf16, tag="efch")
                nc.gpsimd.memset(ef[:, :], 0.0)
                nc.sync.dma_start(ef[:KA, :n * P],
                                  ef_ext[:, t0 * P:(t0 + n) * P])
                hh = chunkp.tile([P, CH, F], f32, tag="hch")
                nc.sync.dma_start(
                    hh[:, :n, :],
                    h_ext[t0 * P:(t0 + n) * P, :].rearrange(
                        "(a p) f -> p a f", p=P))
                dd = chunkp.tile([P, CH, 1], f32, tag="dch")
                nc.sync.dma_start(
                    dd[:, :n, :],
                    d_ext[t0 * P:(t0 + n) * P, :].rearrange(
                        "(a p) o -> p a o", p=P))
                ef_ch[ci], h_ch[ci], d_ch[ci] = ef, hh, dd

            # flat tile order; windows are contiguous runs of tiles
            tile_win = np.repeat(np.arange(W), Tw)
            win_last = np.concatenate([[0], np.cumsum(Tw)])[1:] - 1
            win_first = np.concatenate([[0], np.cumsum(Tw)])[:-1]

            acc_of_win = {}
            for tp in range(T // 2):
                t0 = 2 * tp
                ci, si = divmod(t0, CH)
                if ef_ch[ci] is None:
                    load_chunk(ci)

                wps = wpsp.tile([P, 8 * P], f32, tag="wps")
                e12 = workp.tile([P, 8 * P], bf16, tag="e12")
                pay = workp.tile([P, 8 * P], bf16, tag="pay")
                for pi in range(2):
                    ef_t = ef_ch[ci][:, (si + pi) * P:(si + pi + 1) * P]
                    nc.tensor.matmul(wps[:, pi * 512:(pi + 1) * 512], ef_t,
                                     wcat[:], start=True, stop=True,
                                     skip_group_check=True)
                # e12 = wps * h_broadcast (per tile: AP is limited to
                # 3 free dims, the pairwise view would need 4)
                for pi in range(2):
                    h1 = (h_ch[ci][:, si + pi, :]
                          .unsqueeze(1).unsqueeze(3)
                          .broadcast_to([P, 2, F, F]))
                    nc.vector.tensor_tensor(
                        e12[:, pi * 512:(pi + 1) * 512].rearrange(
                            "p (a i j) -> p a i j", a=2, i=F),
                        wps[:, pi * 512:(pi + 1) * 512].rearrange(
                            "p (a i j) -> p a i j", a=2, i=F),
                        h1, mult)
                e12v = e12[:].rearrange("p (t a c) -> p t a c", t=2, a=2)
                payv = pay[:].rearrange("p (t a c) -> p t a c", t=2, a=2)
                nc.scalar.activation(payv[:, :, 1, :], e12v[:, :, 1, :],
                                     mybir.ActivationFunctionType.Exp)
                nc.gpsimd.tensor_tensor(payv[:, :, 0, :], e12v[:, :, 0, :],
                                        payv[:, :, 1, :], mult)

                for pi in range(2):
                    t = t0 + pi
                    w = int(tile_win[t])
                    d_t = d_ch[ci][:, si + pi, :]
                    oh = workp.tile([P, P], bf16, tag="oh")
                    nc.vector.tensor_scalar(oh[:], iota_b[:], d_t, None,
                                            mybir.AluOpType.is_equal)
                    if w not in acc_of_win:
                        acc = accp.tile([P, 4 * P], f32, tag="acc")
                        acc_of_win[w] = acc
                    acc = acc_of_win[w]
                    nc.tensor.matmul(acc[:],
                                     oh[:],
                                     pay[:, pi * 512:(pi + 1) * 512],
                                     start=(t == win_first[w]),
                                     stop=(t == win_last[w]),
                                     skip_group_check=True)
                    if t == win_last[w]:
                        # stash num/den; division batched at the end
                        nc.scalar.copy(nums[:, w, :], acc[:, :2 * P])
                        nc.scalar.copy(dens[:, w, :], acc[:, 2 * P:])
                        del acc_of_win[w]
                if si + 1 == CH - 1:
                    ef_ch[ci] = h_ch[ci] = d_ch[ci] = None

            # tail: out[n,j] = sum_i num/den over all windows at once
            rden = tailp.tile([P, W, 2 * P], f32, tag="rden")
            nc.scalar.activation(rden[:], dens[:],
                                 mybir.ActivationFunctionType.Ln,
                                 bias=eps[:])
            nc.scalar.activation(rden[:], rden[:],
                                 mybir.ActivationFunctionType.Exp,
                                 scale=-1.0)
            ft = tailp.tile([P, W, 2 * P], f32, tag="ft")
            nc.vector.tensor_tensor(ft[:], nums[:], rden[:], mult)
            outw = tailp.tile([P, W, F], f32, tag="outw")
            nc.vector.tensor_reduce(
                outw[:],
                ft[:].rearrange("p w (i j) -> p w j i", i=F),
                mybir.AxisListType.X, mybir.AluOpType.add)
            nc.sync.dma_start(
                out_ext[:].rearrange("(w p) f -> p w f", p=P), outw[:])
    nc.compile()
    return nc


TRACE = False          # set True (e.g. from test.py) to capture a profile
TRACE_DIR = None       # where to keep NTFF/perfetto artifacts
LAST_RESULT = None     # BassKernelResults of the last run (for profiling)


def kernel(feat, efeat, W_msg, b_msg, W_attn, b_attn, src, dst):
    global LAST_RESULT
    from concourse.bass_utils import run_bass_kernel_spmd

    in_maps, Tw, T = _prep(feat, efeat, W_msg, b_msg, W_attn, b_attn,
                           src, dst)
    nc = _build(Tw, T)
    res = run_bass_kernel_spmd(nc, in_maps, core_ids=list(range(C)),
                               trace=TRACE, tmpdir=TRACE_DIR)
    LAST_RESULT = res
    out = np.empty((N_NODES, F), np.float32)
    for c in range(C):
        out[c * NPC:(c + 1) * NPC] = res.results[c]["out"][:NPC]
    return out


# revision 23
# speedup vs baseline: 2.7566x; 1.1110x over previous
"""AMPNNConv distributed Trainium2 kernel.

Math (reformulated from the reference, numerically equivalent):
    w_m = efeat @ W_msg + b_msg          [E, 16*16]
    w_a = efeat @ W_attn + b_attn        [E, 16*16]
    h   = feat[src]                      [E, 16]   (broadcast over out dim)
    ex  = exp(w_a * h)                   (no max-subtraction: |w_a*h| <~ 8,
                                          exp is f32-safe; softmax is
                                          shift-invariant)
    num[n] = sum_{e: dst[e]=n} (w_m*h) * ex
    den[n] = sum_{e: dst[e]=n} ex
    out[n, j] = sum_i num[n,i,j] / den[n,i,j]

Sharding: edges sorted by dst on host; core c owns nodes
[c*3750, (c+1)*3750) so every segment-sum is core-local (no collectives).
Within a core, edges are bucketed into 30 windows of 128 destination
nodes; each 128-edge tile targets one window and the segment-sum is a
one-hot matmul accumulated in PSUM.

Performance notes (measured on TRN2):
- bf16 matmuls with K padded to 128 stream at ~216ns/512 cols; K<128
  runs 2x slower, f32 runs LOW_HIGH 4-pass.  The K-pad is free: only the
  constant Wcat operand needs zeros in rows 17..127 (garbage * 0 = 0),
  so the per-tile efeatT SBUF rows 17..127 are never written.
- LDWEIGHTS overlaps with in-flight matmuls (64-deep PE queue).
- Elementwise ops process 2 tiles per instruction to amortize the
  per-op overhead (SBUF read-write bubble errata).
- num/den are stashed to SBUF per window; Ln/Exp division runs once at
  the end (2 ACT table loads instead of 2 per window).
"""

import numpy as np

N_NODES = 30000
N_EDGES = 300000
F = 16              # in = out = edge_dim
C = 8               # cores
NPC = N_NODES // C  # nodes per core = 3750
P = 128
W = (NPC + P - 1) // P  # windows per core = 30
NPAD = W * P            # padded nodes per core = 3840
KA = 17                 # edge_dim + 1 (bias row)


def _prep(feat, efeat, W_msg, b_msg, W_attn, b_attn, src, dst):
    """Host-side shard/sort/pad. Returns (in_maps, Tw, T)."""
    import ml_dtypes
    f32 = np.float32
    bf16 = ml_dtypes.bfloat16
    order = np.argsort(dst, kind="stable")
    dsts = dst[order].astype(np.int64)
    core_of = dsts // NPC
    nloc = dsts - core_of * NPC
    win = nloc // P
    wloc = nloc % P

    # tiles per window: max over cores, >= 1
    cnt = np.zeros((C, W), np.int64)
    np.add.at(cnt, (core_of, win), 1)
    Tw = np.maximum(1, -(-cnt.max(axis=0) // P)).astype(np.int64)
    if Tw.sum() % 2:
        Tw[-1] += 1          # keep T even (elementwise ops pair tiles)
    T = int(Tw.sum())
    wb = np.concatenate([[0], np.cumsum(Tw)])[:-1]  # window tile base

    # [128, 512] combined weights: rows 0..15 = W, row 16 = bias,
    # rows 17..127 = zeros (K-pad so matmuls run at K=128 speed)
    Wcat = np.zeros((P, 4 * P), f32)
    Wcat[:F] = np.concatenate([W_msg, W_attn], axis=1)
    Wcat[F] = np.concatenate([b_msg, b_attn])

    in_maps = []
    for c in range(C):
        m = core_of == c
        e_idx = order[m]          # original edge ids, sorted by local node
        w_c = win[m]
        wl = wloc[m]
        cc = cnt[c]
        run_starts = np.concatenate([[0], np.cumsum(cc)])[:-1]
        rank = np.arange(m.sum()) - np.repeat(run_starts, cc)
        slot = (wb[w_c] * P + rank).astype(np.int64)

        efT = np.zeros((KA, T * P), f32)
        h = np.zeros((T * P, F), f32)
        oh = np.zeros((T * P, P), bf16)
        efT[:F, slot] = efeat[e_idx].T
        efT[F, slot] = 1.0
        h[slot] = feat[src[e_idx]]
        oh[slot, wl] = 1.0
        in_maps.append({"efeatT": efT.astype(bf16), "h": h, "oh": oh,
                        "wcat": Wcat.astype(bf16)})
    return in_maps, Tw, T


def _build(Tw, T):
    import concourse.bass as bass
    import concourse.mybir as mybir
    from concourse import bacc, tile

    f32 = mybir.dt.float32
    bf16 = mybir.dt.bfloat16
    i32 = mybir.dt.int32
    mult = mybir.AluOpType.mult

    nc = bacc.Bacc(None, target_bir_lowering=False)
    ef_ext = nc.declare_dram_parameter("efeatT", [KA, T * P], bf16,
                                       isOutput=False)
    h_ext = nc.declare_dram_parameter("h", [T * P, F], f32, isOutput=False)
    oh_ext = nc.declare_dram_parameter("oh", [T * P, P], bf16,
                                       isOutput=False)
    w_ext = nc.declare_dram_parameter("wcat", [P, 4 * P], bf16,
                                      isOutput=False)
    out_ext = nc.declare_dram_parameter("out", [NPAD, F], f32, isOutput=True)

    CH = 8  # tiles per DMA chunk (must be even)

    with tile.TileContext(nc) as tc:
        with (
            tc.tile_pool(name="const", bufs=1) as constp,
            tc.tile_pool(name="chunk", bufs=3) as chunkp,
            tc.tile_pool(name="work", bufs=4) as workp,
            tc.tile_pool(name="stash", bufs=1) as stashp,
            tc.tile_pool(name="tail", bufs=1) as tailp,
            tc.tile_pool(name="wps", bufs=2, space=bass.MemorySpace.PSUM) as wpsp,
            tc.tile_pool(name="acc", bufs=2, space=bass.MemorySpace.PSUM) as accp,
        ):
            wcat = constp.tile([P, 4 * P], bf16, tag="wcat")
            nc.sync.dma_start(wcat[:], w_ext[:])
            eps = constp.tile([P, 1], f32, tag="eps")
            nc.vector.memset(eps[:], 1e-30)

            # per-window num/den stash in SBUF (f32)
            nums = stashp.tile([P, W, 2 * P], f32, tag="nums")
            dens = stashp.tile([P, W, 2 * P], f32, tag="dens")

            n_chunks = (T + CH - 1) // CH
            ef_ch = [None] * n_chunks
            h_ch = [None] * n_chunks
            d_ch = [None] * n_chunks

            def load_chunk(ci):
                t0 = ci * CH
                n = min(CH, T - t0)
                # rows 17..127 must be finite (NaN*0=NaN in the matmul);
                # values are irrelevant since Wcat rows 17..127 are zero
                ef = chunkp.tile([P, CH * P], bf16, tag="efch")
                nc.gpsimd.memset(ef[:, :], 0.0)
                nc.sync.dma_start(ef[:KA, :n * P],
                                  ef_ext[:, t0 * P:(t0 + n) * P])
                hh = chunkp.tile([P, CH, F], f32, tag="hch")
                nc.sync.dma_start(
                    hh[:, :n, :],
                    h_ext[t0 * P:(t0 + n) * P, :].rearrange(
                        "(a p) f -> p a f", p=P))
                dd = chunkp.tile([P, CH, P], bf16, tag="dch")
                nc.sync.dma_start(
                    dd[:, :n, :],
                    oh_ext[t0 * P:(t0 + n) * P, :].rearrange(
                        "(a p) o -> p a o", p=P))
                ef_ch[ci], h_ch[ci], d_ch[ci] = ef, hh, dd

            # flat tile order; windows are contiguous runs of tiles
            tile_win = np.repeat(np.arange(W), Tw)
            win_last = np.concatenate([[0], np.cumsum(Tw)])[1:] - 1
            win_first = np.concatenate([[0], np.cumsum(Tw)])[:-1]

            acc_of_win = {}
            for tp in range(T // 2):
                t0 = 2 * tp
                ci, si = divmod(t0, CH)
                if ef_ch[ci] is None:
                    load_chunk(ci)

                wps = wpsp.tile([P, 8 * P], f32, tag="wps")
                e12 = workp.tile([P, 8 * P], bf16, tag="e12")
                pay = workp.tile([P, 8 * P], bf16, tag="pay")
                for pi in range(2):
                    ef_t = ef_ch[ci][:, (si + pi) * P:(si + pi + 1) * P]
                    nc.tensor.matmul(wps[:, pi * 512:(pi + 1) * 512], ef_t,
                                     wcat[:], start=True, stop=True,
                                     skip_group_check=True)
                # e12 = wps * h_broadcast (per tile: AP is limited to
                # 3 free dims, the pairwise view would need 4)
                for pi in range(2):
                    h1 = (h_ch[ci][:, si + pi, :]
                          .unsqueeze(1).unsqueeze(3)
                          .broadcast_to([P, 2, F, F]))
                    nc.vector.tensor_tensor(
                        e12[:, pi * 512:(pi + 1) * 512].rearrange(
                            "p (a i j) -> p a i j", a=2, i=F),
                        wps[:, pi * 512:(pi + 1) * 512].rearrange(
                            "p (a i j) -> p a i j", a=2, i=F),
                        h1, mult)
                e12v = e12[:].rearrange("p (t a c) -> p t a c", t=2, a=2)
                payv = pay[:].rearrange("p (t a c) -> p t a c", t=2, a=2)
                nc.scalar.activation(payv[:, :, 1, :], e12v[:, :, 1, :],
                                     mybir.ActivationFunctionType.Exp)
                nc.gpsimd.tensor_tensor(payv[:, :, 0, :], e12v[:, :, 0, :],
                                        payv[:, :, 1, :], mult)

                for pi in range(2):
                    t = t0 + pi
                    w = int(tile_win[t])
                    if w not in acc_of_win:
                        acc = accp.tile([P, 4 * P], f32, tag="acc")
                        acc_of_win[w] = acc
                    acc = acc_of_win[w]
                    nc.tensor.matmul(acc[:],
                                     d_ch[ci][:, si + pi, :],
                                     pay[:, pi * 512:(pi + 1) * 512],
                                     start=(t == win_first[w]),
                                     stop=(t == win_last[w]),
                                     skip_group_check=True)
                    if t == win_last[w]:
                        # stash num/den; division batched at the end
                        nc.scalar.copy(nums[:, w, :], acc[:, :2 * P])
                        nc.scalar.copy(dens[:, w, :], acc[:, 2 * P:])
                        del acc_of_win[w]
                if si + 1 == CH - 1:
                    ef_ch[ci] = h_ch[ci] = d_ch[ci] = None

            # tail: out[n,j] = sum_i num/den over all windows at once
            rden = tailp.tile([P, W, 2 * P], f32, tag="rden")
            nc.scalar.activation(rden[:], dens[:],
                                 mybir.ActivationFunctionType.Ln,
                                 bias=eps[:])
            nc.scalar.activation(rden[:], rden[:],
                                 mybir.ActivationFunctionType.Exp,
                                 scale=-1.0)
            ft = tailp.tile([P, W, 2 * P], f32, tag="ft")
            nc.vector.tensor_tensor(ft[:], nums[:], rden[:], mult)
            outw = tailp.tile([P, W, F], f32, tag="outw")
            nc.vector.tensor_reduce(
                outw[:],
                ft[:].rearrange("p w (i j) -> p w j i", i=F),
                mybir.AxisListType.X, mybir.AluOpType.add)
            nc.sync.dma_start(
                out_ext[:].rearrange("(w p) f -> p w f", p=P), outw[:])
    nc.compile()
    return nc


TRACE = False          # set True (e.g. from test.py) to capture a profile
TRACE_DIR = None       # where to keep NTFF/perfetto artifacts
LAST_RESULT = None     # BassKernelResults of the last run (for profiling)


def kernel(feat, efeat, W_msg, b_msg, W_attn, b_attn, src, dst):
    global LAST_RESULT
    from concourse.bass_utils import run_bass_kernel_spmd

    in_maps, Tw, T = _prep(feat, efeat, W_msg, b_msg, W_attn, b_attn,
                           src, dst)
    nc = _build(Tw, T)
    res = run_bass_kernel_spmd(nc, in_maps, core_ids=list(range(C)),
                               trace=TRACE, tmpdir=TRACE_DIR)
    LAST_RESULT = res
    out = np.empty((N_NODES, F), np.float32)
    for c in range(C):
        out[c * NPC:(c + 1) * NPC] = res.results[c]["out"][:NPC]
    return out


# revision 28
# speedup vs baseline: 3.2399x; 1.1753x over previous
"""AMPNNConv distributed Trainium2 kernel.

Math (reformulated from the reference, numerically equivalent):
    w_m = efeat @ W_msg + b_msg          [E, 16*16]
    w_a = efeat @ W_attn + b_attn        [E, 16*16]
    h   = feat[src]                      [E, 16]   (broadcast over out dim)
    ex  = exp(w_a * h)                   (no max-subtraction: |w_a*h| <~ 8,
                                          exp is f32-safe; softmax is
                                          shift-invariant)
    num[n] = sum_{e: dst[e]=n} (w_m*h) * ex
    den[n] = sum_{e: dst[e]=n} ex
    out[n, j] = sum_i num[n,i,j] / den[n,i,j]

Sharding: edges sorted by dst on host; core c owns nodes
[c*3750, (c+1)*3750) so every segment-sum is core-local (no collectives).
Within a core, edges are bucketed into 30 windows of 128 destination
nodes; each 128-edge tile targets one window and the segment-sum is a
one-hot matmul accumulated in PSUM.

Performance notes (measured on TRN2):
- bf16 matmuls with K padded to 128 stream at ~216ns/512 cols; K<128
  runs 2x slower, f32 runs LOW_HIGH 4-pass.  The K-pad is free: only the
  constant Wcat operand needs zeros in rows 17..127 (garbage * 0 = 0),
  so the per-tile efeatT SBUF rows 17..127 are never written.
- LDWEIGHTS overlaps with in-flight matmuls (64-deep PE queue).
- Elementwise ops process 2 tiles per instruction to amortize the
  per-op overhead (SBUF read-write bubble errata).
- num/den are stashed to SBUF per window; Ln/Exp division runs once at
  the end (2 ACT table loads instead of 2 per window).
"""

import numpy as np

N_NODES = 30000
N_EDGES = 300000
F = 16              # in = out = edge_dim
C = 8               # cores
NPC = N_NODES // C  # nodes per core = 3750
P = 128
W = (NPC + P - 1) // P  # windows per core = 30
NPAD = W * P            # padded nodes per core = 3840
KA = 17                 # edge_dim + 1 (bias row)


def _prep(feat, efeat, W_msg, b_msg, W_attn, b_attn, src, dst):
    """Host-side shard/sort/pad. Returns (in_maps, Tw, T)."""
    import ml_dtypes
    f32 = np.float32
    bf16 = ml_dtypes.bfloat16
    order = np.argsort(dst, kind="stable")
    dsts = dst[order].astype(np.int64)
    core_of = dsts // NPC
    nloc = dsts - core_of * NPC
    win = nloc // P
    wloc = nloc % P

    # tiles per window: max over cores, >= 1
    cnt = np.zeros((C, W), np.int64)
    np.add.at(cnt, (core_of, win), 1)
    Tw = np.maximum(1, -(-cnt.max(axis=0) // P)).astype(np.int64)
    if Tw.sum() % 2:
        Tw[-1] += 1          # keep T even (elementwise ops pair tiles)
    T = int(Tw.sum())
    wb = np.concatenate([[0], np.cumsum(Tw)])[:-1]  # window tile base

    # [128, 512] combined weights: rows 0..15 = W, row 16 = bias,
    # rows 17..127 = zeros (K-pad so matmuls run at K=128 speed)
    Wcat = np.zeros((P, 4 * P), f32)
    Wcat[:F] = np.concatenate([W_msg, W_attn], axis=1)
    Wcat[F] = np.concatenate([b_msg, b_attn])

    in_maps = []
    for c in range(C):
        m = core_of == c
        e_idx = order[m]          # original edge ids, sorted by local node
        w_c = win[m]
        wl = wloc[m]
        cc = cnt[c]
        run_starts = np.concatenate([[0], np.cumsum(cc)])[:-1]
        rank = np.arange(m.sum()) - np.repeat(run_starts, cc)
        slot = (wb[w_c] * P + rank).astype(np.int64)

        efT = np.zeros((KA, T * P), f32)
        h = np.zeros((T * P, F), f32)
        oh = np.zeros((T * P, P), bf16)
        efT[:F, slot] = efeat[e_idx].T
        efT[F, slot] = 1.0
        h[slot] = feat[src[e_idx]]
        oh[slot, wl] = 1.0
        # pre-transpose per 128-edge tile so every DMA is contiguous
        # per partition: [T*P, x] -> [P, T, x]
        h_t = np.ascontiguousarray(
            h.reshape(T, P, F).transpose(1, 0, 2))
        oh_t = np.ascontiguousarray(
            oh.reshape(T, P, P).transpose(1, 0, 2))
        in_maps.append({"efeatT": efT.astype(bf16), "h": h_t, "oh": oh_t,
                        "wcat": Wcat.astype(bf16)})
    return in_maps, Tw, T


def _build(Tw, T):
    import concourse.bass as bass
    import concourse.mybir as mybir
    from concourse import bacc, tile

    f32 = mybir.dt.float32
    bf16 = mybir.dt.bfloat16
    i32 = mybir.dt.int32
    mult = mybir.AluOpType.mult

    nc = bacc.Bacc(None, target_bir_lowering=False)
    ef_ext = nc.declare_dram_parameter("efeatT", [KA, T * P], bf16,
                                       isOutput=False)
    h_ext = nc.declare_dram_parameter("h", [P, T, F], f32, isOutput=False)
    oh_ext = nc.declare_dram_parameter("oh", [P, T, P], bf16,
                                       isOutput=False)
    w_ext = nc.declare_dram_parameter("wcat", [P, 4 * P], bf16,
                                      isOutput=False)
    out_ext = nc.declare_dram_parameter("out", [NPAD, F], f32, isOutput=True)

    CH = 8  # tiles per DMA chunk (must be even)

    with tile.TileContext(nc) as tc:
        with (
            tc.tile_pool(name="const", bufs=1) as constp,
            tc.tile_pool(name="chunk", bufs=3) as chunkp,
            tc.tile_pool(name="work", bufs=4) as workp,
            tc.tile_pool(name="stash", bufs=1) as stashp,
            tc.tile_pool(name="tail", bufs=1) as tailp,
            tc.tile_pool(name="wps", bufs=3, space=bass.MemorySpace.PSUM) as wpsp,
            tc.tile_pool(name="acc", bufs=2, space=bass.MemorySpace.PSUM) as accp,
        ):
            wcat = constp.tile([P, 4 * P], bf16, tag="wcat")
            nc.sync.dma_start(wcat[:], w_ext[:])
            eps = constp.tile([P, 1], f32, tag="eps")
            nc.vector.memset(eps[:], 1e-30)

            # per-window num/den stash in SBUF (f32)
            nums = stashp.tile([P, W, 2 * P], f32, tag="nums")
            dens = stashp.tile([P, W, 2 * P], f32, tag="dens")

            n_chunks = (T + CH - 1) // CH
            ef_ch = [None] * n_chunks
            h_ch = [None] * n_chunks
            d_ch = [None] * n_chunks

            def load_chunk(ci):
                t0 = ci * CH
                n = min(CH, T - t0)
                # rows 17..127 must be finite (NaN*0=NaN in the matmul);
                # values are irrelevant since Wcat rows 17..127 are zero
                ef = chunkp.tile([P, CH * P], bf16, tag="efch")
                nc.gpsimd.memset(ef[:, :], 0.0)
                nc.sync.dma_start(ef[:KA, :n * P],
                                  ef_ext[:, t0 * P:(t0 + n) * P])
                hh = chunkp.tile([P, CH, F], f32, tag="hch")
                nc.sync.dma_start(hh[:, :n, :], h_ext[:, t0:t0 + n, :])
                dd = chunkp.tile([P, CH, P], bf16, tag="dch")
                nc.sync.dma_start(dd[:, :n, :], oh_ext[:, t0:t0 + n, :])
                ef_ch[ci], h_ch[ci], d_ch[ci] = ef, hh, dd

            # flat tile order; windows are contiguous runs of tiles
            tile_win = np.repeat(np.arange(W), Tw)
            win_last = np.concatenate([[0], np.cumsum(Tw)])[1:] - 1
            win_first = np.concatenate([[0], np.cumsum(Tw)])[:-1]

            acc_of_win = {}
            for tp in range(T // 2):
                t0 = 2 * tp
                ci, si = divmod(t0, CH)
                if ef_ch[ci] is None:
                    load_chunk(ci)

                wps = wpsp.tile([P, 8 * P], f32, tag="wps")
                e12 = workp.tile([P, 8 * P], bf16, tag="e12")
                pay = workp.tile([P, 8 * P], bf16, tag="pay")
                for pi in range(2):
                    ef_t = ef_ch[ci][:, (si + pi) * P:(si + pi + 1) * P]
                    nc.tensor.matmul(wps[:, pi * 512:(pi + 1) * 512], ef_t,
                                     wcat[:], start=True, stop=True,
                                     skip_group_check=True)
                # e12 = wps * h_broadcast (per tile: AP is limited to
                # 3 free dims, the pairwise view would need 4)
                for pi in range(2):
                    h1 = (h_ch[ci][:, si + pi, :]
                          .unsqueeze(1).unsqueeze(3)
                          .broadcast_to([P, 2, F, F]))
                    nc.vector.tensor_tensor(
                        e12[:, pi * 512:(pi + 1) * 512].rearrange(
                            "p (a i j) -> p a i j", a=2, i=F),
                        wps[:, pi * 512:(pi + 1) * 512].rearrange(
                            "p (a i j) -> p a i j", a=2, i=F),
                        h1, mult)
                e12v = e12[:].rearrange("p (t a c) -> p t a c", t=2, a=2)
                payv = pay[:].rearrange("p (t a c) -> p t a c", t=2, a=2)
                nc.scalar.activation(payv[:, :, 1, :], e12v[:, :, 1, :],
                                     mybir.ActivationFunctionType.Exp)
                nc.vector.tensor_tensor(payv[:, :, 0, :], e12v[:, :, 0, :],
                                        payv[:, :, 1, :], mult)

                for pi in range(2):
                    t = t0 + pi
                    w = int(tile_win[t])
                    if w not in acc_of_win:
                        acc = accp.tile([P, 4 * P], f32, tag="acc")
                        acc_of_win[w] = acc
                    acc = acc_of_win[w]
                    nc.tensor.matmul(acc[:],
                                     d_ch[ci][:, si + pi, :],
                                     pay[:, pi * 512:(pi + 1) * 512],
                                     start=(t == win_first[w]),
                                     stop=(t == win_last[w]),
                                     skip_group_check=True)
                    if t == win_last[w]:
                        # stash num/den; division batched at the end
                        nc.scalar.copy(nums[:, w, :], acc[:, :2 * P])
                        nc.scalar.copy(dens[:, w, :], acc[:, 2 * P:])
                        del acc_of_win[w]
                if si + 1 == CH - 1:
                    ef_ch[ci] = h_ch[ci] = d_ch[ci] = None

            # tail: out[n,j] = sum_i num/den over all windows at once
            rden = tailp.tile([P, W, 2 * P], f32, tag="rden")
            nc.scalar.activation(rden[:], dens[:],
                                 mybir.ActivationFunctionType.Ln,
                                 bias=eps[:])
            nc.scalar.activation(rden[:], rden[:],
                                 mybir.ActivationFunctionType.Exp,
                                 scale=-1.0)
            ft = tailp.tile([P, W, 2 * P], f32, tag="ft")
            nc.vector.tensor_tensor(ft[:], nums[:], rden[:], mult)
            outw = tailp.tile([P, W, F], f32, tag="outw")
            nc.vector.tensor_reduce(
                outw[:],
                ft[:].rearrange("p w (i j) -> p w j i", i=F),
                mybir.AxisListType.X, mybir.AluOpType.add)
            nc.sync.dma_start(
                out_ext[:].rearrange("(w p) f -> p w f", p=P), outw[:])
    nc.compile()
    return nc


TRACE = False          # set True (e.g. from test.py) to capture a profile
TRACE_DIR = None       # where to keep NTFF/perfetto artifacts
LAST_RESULT = None     # BassKernelResults of the last run (for profiling)


def kernel(feat, efeat, W_msg, b_msg, W_attn, b_attn, src, dst):
    global LAST_RESULT
    from concourse.bass_utils import run_bass_kernel_spmd

    in_maps, Tw, T = _prep(feat, efeat, W_msg, b_msg, W_attn, b_attn,
                           src, dst)
    nc = _build(Tw, T)
    res = run_bass_kernel_spmd(nc, in_maps, core_ids=list(range(C)),
                               trace=TRACE, tmpdir=TRACE_DIR)
    LAST_RESULT = res
    out = np.empty((N_NODES, F), np.float32)
    for c in range(C):
        out[c * NPC:(c + 1) * NPC] = res.results[c]["out"][:NPC]
    return out


# revision 35
# speedup vs baseline: 3.4381x; 1.0612x over previous
"""AMPNNConv distributed Trainium2 kernel.

Math (reformulated from the reference, numerically equivalent):
    w_m = efeat @ W_msg + b_msg          [E, 16*16]
    w_a = efeat @ W_attn + b_attn        [E, 16*16]
    h   = feat[src]                      [E, 16]   (broadcast over out dim)
    ex  = exp(w_a * h)                   (no max-subtraction: |w_a*h| <~ 8,
                                          exp is f32-safe; softmax is
                                          shift-invariant)
    num[n] = sum_{e: dst[e]=n} (w_m*h) * ex
    den[n] = sum_{e: dst[e]=n} ex
    out[n, j] = sum_i num[n,i,j] / den[n,i,j]

Sharding: edges sorted by dst on host; core c owns nodes
[c*3750, (c+1)*3750) so every segment-sum is core-local (no collectives).
Within a core, edges are bucketed into 30 windows of 128 destination
nodes; each 128-edge tile targets one window and the segment-sum is a
one-hot matmul accumulated in PSUM.

Performance notes (measured on TRN2):
- bf16 matmuls with K padded to 128 stream at ~216ns/512 cols; K<128
  runs 2x slower, f32 runs LOW_HIGH 4-pass.  The K-pad is free: only the
  constant Wcat operand needs zeros in rows 17..127 (garbage * 0 = 0),
  so the per-tile efeatT SBUF rows 17..127 are never written.
- LDWEIGHTS overlaps with in-flight matmuls (64-deep PE queue).
- Elementwise ops process 2 tiles per instruction to amortize the
  per-op overhead (SBUF read-write bubble errata).
- num/den are stashed to SBUF per window; Ln/Exp division runs once at
  the end (2 ACT table loads instead of 2 per window).
"""

import numpy as np

N_NODES = 30000
N_EDGES = 300000
F = 16              # in = out = edge_dim
C = 8               # cores
NPC = N_NODES // C  # nodes per core = 3750
P = 128
W = (NPC + P - 1) // P  # windows per core = 30
NPAD = W * P            # padded nodes per core = 3840
KA = 17                 # edge_dim + 1 (bias row)


def _prep(feat, efeat, W_msg, b_msg, W_attn, b_attn, src, dst):
    """Host-side shard/sort/pad. Returns (in_maps, Tw, T)."""
    import ml_dtypes
    f32 = np.float32
    bf16 = ml_dtypes.bfloat16
    order = np.argsort(dst, kind="stable")
    dsts = dst[order].astype(np.int64)
    core_of = dsts // NPC
    nloc = dsts - core_of * NPC
    win = nloc // P
    wloc = nloc % P

    # tiles per window: max over cores, >= 1
    cnt = np.zeros((C, W), np.int64)
    np.add.at(cnt, (core_of, win), 1)
    Tw = np.maximum(1, -(-cnt.max(axis=0) // P)).astype(np.int64)
    if Tw.sum() % 2:
        Tw[-1] += 1          # keep T even (elementwise ops pair tiles)
    T = int(Tw.sum())
    wb = np.concatenate([[0], np.cumsum(Tw)])[:-1]  # window tile base

    # [128, 512] combined weights: rows 0..15 = W, row 16 = bias,
    # rows 17..127 = zeros (K-pad so matmuls run at K=128 speed)
    Wcat = np.zeros((P, 4 * P), f32)
    Wcat[:F] = np.concatenate([W_msg, W_attn], axis=1)
    Wcat[F] = np.concatenate([b_msg, b_attn])

    in_maps = []
    for c in range(C):
        m = core_of == c
        e_idx = order[m]          # original edge ids, sorted by local node
        w_c = win[m]
        wl = wloc[m]
        cc = cnt[c]
        run_starts = np.concatenate([[0], np.cumsum(cc)])[:-1]
        rank = np.arange(m.sum()) - np.repeat(run_starts, cc)
        slot = (wb[w_c] * P + rank).astype(np.int64)

        efT = np.zeros((KA, T * P), f32)
        h = np.zeros((T * P, F), f32)
        oh = np.zeros((T * P, P), bf16)
        efT[:F, slot] = efeat[e_idx].T
        efT[F, slot] = 1.0
        h[slot] = feat[src[e_idx]]
        oh[slot, wl] = 1.0
        # pre-broadcast h over the msg/attn axis (32 lanes) and
        # pre-transpose per 128-edge tile so every DMA is contiguous
        # per partition: [T*P, x] -> [P, T, x]
        h2 = np.tile(h, (1, 2)).astype(bf16)
        h_t = np.ascontiguousarray(
            h2.reshape(T, P, 2 * F).transpose(1, 0, 2))
        oh_t = np.ascontiguousarray(
            oh.reshape(T, P, P).transpose(1, 0, 2))
        in_maps.append({"efeatT": efT.astype(bf16), "h": h_t, "oh": oh_t,
                        "wcat": Wcat.astype(bf16)})
    return in_maps, Tw, T


def _build(Tw, T):
    import concourse.bass as bass
    import concourse.mybir as mybir
    from concourse import bacc, tile

    f32 = mybir.dt.float32
    bf16 = mybir.dt.bfloat16
    i32 = mybir.dt.int32
    mult = mybir.AluOpType.mult

    nc = bacc.Bacc(None, target_bir_lowering=False)
    ef_ext = nc.declare_dram_parameter("efeatT", [KA, T * P], bf16,
                                       isOutput=False)
    h_ext = nc.declare_dram_parameter("h", [P, T, 2 * F], bf16,
                                      isOutput=False)
    oh_ext = nc.declare_dram_parameter("oh", [P, T, P], bf16,
                                       isOutput=False)
    w_ext = nc.declare_dram_parameter("wcat", [P, 4 * P], bf16,
                                      isOutput=False)
    out_ext = nc.declare_dram_parameter("out", [NPAD, F], f32, isOutput=True)

    CH = 8  # tiles per DMA chunk (must be even)

    with tile.TileContext(nc) as tc:
        with (
            tc.tile_pool(name="const", bufs=1) as constp,
            tc.tile_pool(name="chunk", bufs=3) as chunkp,
            tc.tile_pool(name="work", bufs=4) as workp,
            tc.tile_pool(name="stash", bufs=1) as stashp,
            tc.tile_pool(name="tail", bufs=1) as tailp,
            tc.tile_pool(name="wps", bufs=3, space=bass.MemorySpace.PSUM) as wpsp,
            tc.tile_pool(name="acc", bufs=2, space=bass.MemorySpace.PSUM) as accp,
        ):
            wcat = constp.tile([P, 4 * P], bf16, tag="wcat")
            nc.sync.dma_start(wcat[:], w_ext[:])
            eps = constp.tile([P, 1], f32, tag="eps")
            nc.vector.memset(eps[:], 1e-30)

            # per-window num/den stash in SBUF (f32)
            nums = stashp.tile([P, W, 2 * P], f32, tag="nums")
            dens = stashp.tile([P, W, 2 * P], f32, tag="dens")

            n_chunks = (T + CH - 1) // CH
            ef_ch = [None] * n_chunks
            h_ch = [None] * n_chunks
            d_ch = [None] * n_chunks

            def load_chunk(ci):
                t0 = ci * CH
                n = min(CH, T - t0)
                # rows 17..127 must be finite (NaN*0=NaN in the matmul);
                # values are irrelevant since Wcat rows 17..127 are zero
                ef = chunkp.tile([P, CH * P], bf16, tag="efch")
                nc.gpsimd.memset(ef[:, :], 0.0)
                nc.sync.dma_start(ef[:KA, :n * P],
                                  ef_ext[:, t0 * P:(t0 + n) * P])
                hh = chunkp.tile([P, CH, 2 * F], bf16, tag="hch")
                nc.sync.dma_start(hh[:, :n, :], h_ext[:, t0:t0 + n, :])
                dd = chunkp.tile([P, CH, P], bf16, tag="dch")
                nc.sync.dma_start(dd[:, :n, :], oh_ext[:, t0:t0 + n, :])
                ef_ch[ci], h_ch[ci], d_ch[ci] = ef, hh, dd

            # flat tile order; windows are contiguous runs of tiles
            tile_win = np.repeat(np.arange(W), Tw)
            win_last = np.concatenate([[0], np.cumsum(Tw)])[1:] - 1
            win_first = np.concatenate([[0], np.cumsum(Tw)])[:-1]

            # division tails run in batches so they overlap the main loop
            NBATCH = 4
            bounds = [W * (b + 1) // NBATCH for b in range(NBATCH)]

            def emit_tail(w0, w1):
                nw = w1 - w0
                rden = tailp.tile([P, W // NBATCH + 1, 2 * P], f32,
                                  tag="rden")
                nc.scalar.activation(rden[:, :nw, :], dens[:, w0:w1, :],
                                     mybir.ActivationFunctionType.Ln,
                                     bias=eps[:])
                nc.scalar.activation(rden[:, :nw, :], rden[:, :nw, :],
                                     mybir.ActivationFunctionType.Exp,
                                     scale=-1.0)
                ft = tailp.tile([P, W // NBATCH + 1, 2 * P], f32, tag="ft")
                nc.vector.tensor_tensor(ft[:, :nw, :], nums[:, w0:w1, :],
                                        rden[:, :nw, :], mult)
                outw = tailp.tile([P, W // NBATCH + 1, F], f32, tag="outw")
                nc.vector.tensor_reduce(
                    outw[:, :nw, :],
                    ft[:, :nw, :].rearrange("p w (i j) -> p w j i", i=F),
                    mybir.AxisListType.X, mybir.AluOpType.add)
                nc.sync.dma_start(
                    out_ext[w0 * P:w1 * P, :].rearrange(
                        "(w p) f -> p w f", p=P),
                    outw[:, :nw, :])

            acc_of_win = {}
            done_w = 0
            for tp in range(T // 2):
                t0 = 2 * tp
                ci, si = divmod(t0, CH)
                if ef_ch[ci] is None:
                    load_chunk(ci)

                wps = wpsp.tile([P, 8 * P], f32, tag="wps")
                e12 = workp.tile([P, 8 * P], bf16, tag="e12")
                pay = workp.tile([P, 8 * P], bf16, tag="pay")
                for pi in range(2):
                    ef_t = ef_ch[ci][:, (si + pi) * P:(si + pi + 1) * P]
                    nc.tensor.matmul(wps[:, pi * 512:(pi + 1) * 512], ef_t,
                                     wcat[:], start=True, stop=True,
                                     skip_group_check=True)
                # e12 = wps * h_broadcast: one op per pair (h comes
                # pre-broadcast over the msg/attn axis, so the AP stays
                # within 3 free dims)
                hb = (h_ch[ci][:, si:si + 2, :].unsqueeze(3)
                      .broadcast_to([P, 2, 2 * F, F]))
                nc.vector.tensor_tensor(
                    e12[:].rearrange("p (t ai j) -> p t ai j",
                                     t=2, ai=2 * F),
                    wps[:].rearrange("p (t ai j) -> p t ai j",
                                     t=2, ai=2 * F),
                    hb, mult)
                e12v = e12[:].rearrange("p (t a c) -> p t a c", t=2, a=2)
                payv = pay[:].rearrange("p (t a c) -> p t a c", t=2, a=2)
                nc.scalar.activation(payv[:, :, 1, :], e12v[:, :, 1, :],
                                     mybir.ActivationFunctionType.Exp)
                nc.gpsimd.tensor_tensor(payv[:, :, 0, :], e12v[:, :, 0, :],
                                        payv[:, :, 1, :], mult)

                for pi in range(2):
                    t = t0 + pi
                    w = int(tile_win[t])
                    if w not in acc_of_win:
                        acc = accp.tile([P, 4 * P], f32, tag="acc")
                        acc_of_win[w] = acc
                    acc = acc_of_win[w]
                    nc.tensor.matmul(acc[:],
                                     d_ch[ci][:, si + pi, :],
                                     pay[:, pi * 512:(pi + 1) * 512],
                                     start=(t == win_first[w]),
                                     stop=(t == win_last[w]),
                                     skip_group_check=True)
                    if t == win_last[w]:
                        # stash num/den; division batched per tail-batch
                        nc.scalar.copy(nums[:, w, :], acc[:, :2 * P])
                        nc.scalar.copy(dens[:, w, :], acc[:, 2 * P:])
                        del acc_of_win[w]
                        if w + 1 in bounds:
                            emit_tail(done_w, w + 1)
                            done_w = w + 1
                if si + 1 == CH - 1:
                    ef_ch[ci] = h_ch[ci] = d_ch[ci] = None
    nc.compile()
    return nc


TRACE = False          # set True (e.g. from test.py) to capture a profile
TRACE_DIR = None       # where to keep NTFF/perfetto artifacts
LAST_RESULT = None     # BassKernelResults of the last run (for profiling)


def kernel(feat, efeat, W_msg, b_msg, W_attn, b_attn, src, dst):
    global LAST_RESULT
    from concourse.bass_utils import run_bass_kernel_spmd

    in_maps, Tw, T = _prep(feat, efeat, W_msg, b_msg, W_attn, b_attn,
                           src, dst)
    nc = _build(Tw, T)
    res = run_bass_kernel_spmd(nc, in_maps, core_ids=list(range(C)),
                               trace=TRACE, tmpdir=TRACE_DIR)
    LAST_RESULT = res
    out = np.empty((N_NODES, F), np.float32)
    for c in range(C):
        out[c * NPC:(c + 1) * NPC] = res.results[c]["out"][:NPC]
    return out
